# revision 1
# baseline (speedup 1.0000x reference)
"""Trainium2 Bass kernel for batched windowed multi-head attention.

Shapes: x (8, 64, 256, 512) f32, H=8 heads, D=64.
Sharding: data-parallel over batch dim B=8 -> 1 batch row per NeuronCore.
Each core processes 64 windows; per window a full MHA block computed in
fp32r (reduced-mantissa fp32, full-rate on the PE array):
  q/k/v projections, transposed scores = (k q^T) + pos_bias^T + mask^T,
  softmax along the PSUM partition axis: exp on ACT, denominators via a
  ones-column appended to V (so z_aug row 64 = sum_j exp), per-head
  normalization via a K=1 PE broadcast matmul + fast DVE reciprocal +
  one DVE multiply, out = z @ Wp^T + bp.
Windows are software-pipelined: projection chunks of window w+1 are
emitted interleaved with the attention heads of window w to keep the
PE dense (HAM clock stays warm).
"""
import os
import numpy as np

import concourse.bass as bass
import concourse.mybir as mybir
import concourse.tile as tile
from concourse import bacc
from concourse.bass_utils import run_bass_kernel_spmd
from concourse.masks import make_identity

B, W, S, E = 8, 64, 256, 512
H, D = 8, 64
SCALE = D ** -0.5
NCORES = 8
F32 = mybir.dt.float32
F32R = mybir.dt.float32r
AOp = mybir.AluOpType
AF = mybir.ActivationFunctionType


def _emit(nc, tc, ctx, n_w, d):
    """Emit the per-core program: n_w windows of MHA."""
    const = ctx.enter_context(tc.tile_pool(name="const", bufs=1))

    # --- one-time: weights (rounded to fp32r), biases, pos_bias, identity ---
    w_sb = {}
    with tc.tile_pool(name="wstage", bufs=2) as wstage:
        for name in ("wq", "wk", "wv", "wp"):
            st = wstage.tile([128, 4, E], F32, tag="wst", name=f"wst_{name}")
            nc.sync.dma_start(st[:], d[name].rearrange("(ic p) o -> p ic o", p=128))
            t = const.tile([128, 4, E], F32R, tag=name)
            nc.vector.tensor_copy(t[:], st[:])
            w_sb[name] = t

    bqc = const.tile([128, 4], F32)
    nc.sync.dma_start(bqc[:], d["bq"][:])
    bkc = const.tile([128, 4], F32)
    nc.sync.dma_start(bkc[:], d["bk"][:])
    bv_bc = const.tile([128, E], F32)
    nc.sync.dma_start(bv_bc[:], d["bv"][:])
    bp_bc = const.tile([128, E], F32)
    nc.sync.dma_start(bp_bc[:], d["bp"][:])

    # pos_bias TRANSPOSED per head: [128 (j%128), h, jc, i]
    pos_sb = const.tile([128, H, 2, S], F32)
    nc.sync.dma_start(pos_sb[:], d["pos"].rearrange("h (c p) j -> p h c j", p=128))

    ident = const.tile([128, 128], F32)
    make_identity(nc, ident[:])
    ones16 = const.tile([128, 2, 8, 1], F32)
    nc.gpsimd.memset(ones16[:], 1.0)
    sel2_st = const.tile([2, 128], F32)
    nc.sync.dma_start(sel2_st[:], d["sel2"][:])
    sel2 = const.tile([2, 128], F32R)
    nc.vector.tensor_copy(sel2[:], sel2_st[:])

    # --- pools for the per-window pipeline ---
    xnat_p = ctx.enter_context(tc.tile_pool(name="xnat", bufs=2))
    msk_p = ctx.enter_context(tc.tile_pool(name="msk", bufs=2))
    mpb_p = ctx.enter_context(tc.tile_pool(name="mpb", bufs=2))
    xt_p = ctx.enter_context(tc.tile_pool(name="xt", bufs=2))
    qkv_p = ctx.enter_context(tc.tile_pool(name="qkv", bufs=2))
    zt_p = ctx.enter_context(tc.tile_pool(name="zt", bufs=2))
    outs_p = ctx.enter_context(tc.tile_pool(name="outs", bufs=2))
    attn_p = ctx.enter_context(tc.tile_pool(name="attn", bufs=4))
    expt_p = ctx.enter_context(tc.tile_pool(name="expt", bufs=4))
    den_p = ctx.enter_context(tc.tile_pool(name="den", bufs=8))

    ps_pj = ctx.enter_context(tc.tile_pool(name="ps_pj", bufs=3, space="PSUM"))
    ps_sc = ctx.enter_context(tc.tile_pool(name="ps_sc", bufs=2, space="PSUM"))
    ps_z = ctx.enter_context(tc.tile_pool(name="ps_z", bufs=3, space="PSUM"))

    def phase_a(w):
        """Load, transpose, and project window w (dense PE work)."""
        # load x window [256, 512] as [128, (s-chunk, e)]
        xnat = xnat_p.tile([128, 2, E], F32, tag="xn", name=f"xn{w}")
        nc.sync.dma_start(xnat[:], d["x"][w].rearrange("(c p) e -> p c e", p=128))
        # mask^T window: [128 (j%128), jc, i]
        mskT = msk_p.tile([128, 2, S], F32, tag="mk", name=f"mk{w}")
        nc.sync.dma_start(mskT[:], d["mask"][w].rearrange("(c p) j -> p c j", p=128))

        # mask^T + pos_bias^T per head (gpsimd, sbuf only)
        mpbT = mpb_p.tile([128, H, 2, S], F32, tag="mpb", name=f"mpb{w}")
        for h in range(H):
            nc.gpsimd.tensor_tensor(mpbT[:, h], mskT[:], pos_sb[:, h], AOp.add)

        # xT [e, s] via PE transposes: [128 (e%128), (ec, s)]
        xT = xt_p.tile([128, 4, S], F32R, tag="xT", name=f"xT{w}")
        for ec in range(4):
            pt = ps_pj.tile([128, 2, 128], F32, tag="pj", name=f"pt{w}_{ec}")
            for c in range(2):
                nc.tensor.transpose(pt[:, c], xnat[:, c, ec * 128:(ec + 1) * 128], ident[:])
            nc.vector.tensor_copy(xT[:, ec], pt[:])

        # projections: qT/kT [o, s] layout [128 (o%128), (oc, s)]
        qT = qkv_p.tile([128, 4, S], F32R, tag="qT", name=f"qT{w}")
        kT = qkv_p.tile([128, 4, S], F32R, tag="kT", name=f"kT{w}")
        vA = qkv_p.tile([128, 2, H, 65], F32R, tag="vA", name=f"vA{w}")
        nc.vector.tensor_copy(vA[:, :, :, 64:65], ones16[:])

        def qk_chunk(oc, wt, dst, bias):
            p = ps_pj.tile([128, S], F32, tag="pj", name=f"pp{w}_{wt}_{oc}")
            for ic in range(4):
                nc.tensor.matmul(p[:], w_sb[wt][:, ic, oc * 128:(oc + 1) * 128],
                                 xT[:, ic], start=(ic == 0), stop=(ic == 3))
            nc.scalar.activation(dst[:, oc], p[:], AF.Identity,
                                 bias=bias[:, oc:oc + 1])

        def v_chunk(sc):
            pv = ps_pj.tile([128, E], F32, tag="pj", name=f"pv{w}_{sc}")
            for ic in range(4):
                nc.tensor.matmul(pv[:], xT[:, ic, sc * 128:(sc + 1) * 128],
                                 w_sb["wv"][:, ic], start=(ic == 0), stop=(ic == 3))
            nc.vector.scalar_tensor_tensor(
                vA[:, sc, :, 0:64], pv[:].rearrange("p (h o) -> p h o", h=H),
                0.0, bv_bc[:].rearrange("p (h o) -> p h o", h=H),
                AOp.bypass, AOp.add)

        chunks = []
        for oc in range(4):
            chunks.append(lambda oc=oc: qk_chunk(oc, "wq", qT, bqc))
            chunks.append(lambda oc=oc: qk_chunk(oc, "wk", kT, bkc))
        chunks.append(lambda: v_chunk(0))
        chunks.append(lambda: v_chunk(1))
        return (qT, kT, vA, mpbT), chunks

    def phase_b(w, qT, kT, vA, mpbT):
        """Attention + output projection for window w (latency-heavy chain)."""
        # attention per head; zT [e, s] layout [128 (e%128), (hp, s)]
        zT = zt_p.tile([128, 4, S], F32R, tag="zT", name=f"zT{w}")

        def head(h):
            oc, prow = h // 2, (h % 2) * 64
            # transposed scores: [128 (j%128), jc, i]
            sT = ps_sc.tile([128, 2, S], F32, tag="sc", name=f"sT{w}_{h}")
            for jc in range(2):
                nc.tensor.matmul(sT[:, jc],
                                 kT[prow:prow + 64, oc, jc * 128:(jc + 1) * 128],
                                 qT[prow:prow + 64, oc], start=True, stop=True)
            astT = attn_p.tile([128, 2, S], F32, tag="astT", name=f"astT{w}_{h}")
            nc.vector.scalar_tensor_tensor(astT[:], sT[:], 0.0, mpbT[:, h],
                                           AOp.bypass, AOp.add)
            expT = expt_p.tile([128, 2, S], F32R, tag="expT", name=f"expT{w}_{h}")
            nc.scalar.activation(expT[:], astT[:], AF.Exp)
            # z_aug [65, i]: rows 0-63 = v_h^T @ exp cols, row 64 = sum_j exp
            za = ps_z.tile([65, S], F32, tag="zz", name=f"za{w}_{h}")
            for jc in range(2):
                nc.tensor.matmul(za[:], vA[:, jc, h], expT[:, jc],
                                 start=(jc == 0), stop=(jc == 1))
            den = den_p.tile([1, S], F32R, tag="den", name=f"den{w}_{h}")
            nc.scalar.copy(den[:], za[64:65, :])
            den_b = ps_z.tile([64, S], F32, tag="zz", name=f"denb{w}_{h}")
            nc.tensor.matmul(den_b[:], sel2[0:1, 0:64], den[:], start=True, stop=True)
            rec_b = den_p.tile([64, S], F32, tag="recb", name=f"recb{w}_{h}")
            nc.vector.reciprocal_approx_fast(rec_b[:], den_b[:])
            nc.vector.tensor_tensor(zT[prow:prow + 64, h // 2], za[0:64, :],
                                    rec_b[:], AOp.mult)

        def tail():
            # output projection [s, o] natural + bias, then store
            out_sb = outs_p.tile([128, 2, E], F32, tag="osb", name=f"osb{w}")
            for sc in range(2):
                po = ps_pj.tile([128, E], F32, tag="pj", name=f"po{w}_{sc}")
                for ec in range(4):
                    nc.tensor.matmul(po[:], zT[:, ec, sc * 128:(sc + 1) * 128],
                                     w_sb["wp"][:, ec], start=(ec == 0), stop=(ec == 3))
                nc.vector.scalar_tensor_tensor(out_sb[:, sc], po[:], 0.0, bp_bc[:],
                                               AOp.bypass, AOp.add)
            nc.sync.dma_start(d["out"][w].rearrange("(c p) e -> p c e", p=128), out_sb[:])

        return [lambda h=h: head(h) for h in range(H)], tail

    prev = None
    for w in range(n_w):
        cur, chunks = phase_a(w)
        if prev is not None:
            # interleave: one projection chunk of window w between heads of w-1
            heads, tail = phase_b(w - 1, *prev)
            seq = []
            ci = 0
            for hfn in heads:
                if ci < len(chunks):
                    seq.append(chunks[ci]); ci += 1
                seq.append(hfn)
            seq.extend(chunks[ci:])
            seq.append(tail)
            for fn in seq:
                fn()
        else:
            for fn in chunks:
                fn()
        prev = cur
    heads, tail = phase_b(n_w - 1, *prev)
    for fn in heads:
        fn()
    tail()


def _build(n_w):
    nc = bacc.Bacc("TRN2", target_bir_lowering=False, debug=False)
    d = {
        "x": nc.dram_tensor("x", [n_w, S, E], F32, kind="ExternalInput"),
        "mask": nc.dram_tensor("mask", [n_w, S, S], F32, kind="ExternalInput"),
        "pos": nc.dram_tensor("pos", [H, S, S], F32, kind="ExternalInput"),
        "wq": nc.dram_tensor("wq", [E, E], F32, kind="ExternalInput"),
        "wk": nc.dram_tensor("wk", [E, E], F32, kind="ExternalInput"),
        "wv": nc.dram_tensor("wv", [E, E], F32, kind="ExternalInput"),
        "wp": nc.dram_tensor("wp", [E, E], F32, kind="ExternalInput"),
        "bq": nc.dram_tensor("bq", [128, 4], F32, kind="ExternalInput"),
        "bk": nc.dram_tensor("bk", [128, 4], F32, kind="ExternalInput"),
        "bv": nc.dram_tensor("bv", [128, E], F32, kind="ExternalInput"),
        "bp": nc.dram_tensor("bp", [128, E], F32, kind="ExternalInput"),
        "sel2": nc.dram_tensor("sel2", [2, 128], F32, kind="ExternalInput"),
        "out": nc.dram_tensor("out", [n_w, S, E], F32, kind="ExternalOutput"),
    }
    from contextlib import ExitStack
    with tile.TileContext(nc) as tc, ExitStack() as ctx:
        _emit(nc, tc, ctx, n_w, d)
    nc.compile()
    return nc


_NC_CACHE = {}


def _get_nc(n_w):
    if n_w not in _NC_CACHE:
        _NC_CACHE[n_w] = _build(n_w)
    return _NC_CACHE[n_w]


def _host_prep(mask, Wq, bq, Wk, bk, Wv, bv, Wp, bp, pos_bias):
    """Shared (replicated) input tensors, host-side layout prep."""
    f = np.float32
    wq_t = np.ascontiguousarray(Wq.T * SCALE, dtype=f)  # [in, out], SCALE folded
    wk_t = np.ascontiguousarray(Wk.T, dtype=f)
    wv_t = np.ascontiguousarray(Wv.T, dtype=f)
    wp_t = np.ascontiguousarray(Wp.T, dtype=f)
    bq_s = (bq * SCALE).astype(f)
    # bias tiles for qT/kT layout: [128 (o%128), oc, s] broadcast along s
    bq_t = np.ascontiguousarray(bq_s.reshape(4, 128).T)
    bk_t = np.ascontiguousarray(np.asarray(bk, f).reshape(4, 128).T)
    bv_bc = np.ascontiguousarray(np.broadcast_to(np.asarray(bv, f)[None, :], (128, E)))
    bp_bc = np.ascontiguousarray(np.broadcast_to(np.asarray(bp, f)[None, :], (128, E)))
    # transposed mask / pos_bias for the partition-axis softmax layout
    maskt = np.ascontiguousarray(np.asarray(mask, f)[0, :, 0].transpose(0, 2, 1))
    sel2 = np.ascontiguousarray((np.arange(128)[None, :] // 64 == np.arange(2)[:, None]).astype(f))
    post = np.ascontiguousarray(np.asarray(pos_bias, f).transpose(0, 2, 1))
    return {
        "wq": wq_t, "wk": wk_t, "wv": wv_t, "wp": wp_t,
        "bq": bq_t, "bk": bk_t, "bv": bv_bc, "bp": bp_bc,
        "pos": post, "_maskt": maskt,
        "sel2": sel2,
    }


def kernel(x, mask, Wq, bq, Wk, bk, Wv, bv, Wp, bp, pos_bias, _trace=False):
    n_w = int(os.environ.get("KERNEL_NW", W))
    n_cores = NCORES
    x = np.asarray(x, np.float32)
    shared = _host_prep(mask, Wq, bq, Wk, bk, Wv, bv, Wp, bp, pos_bias)
    maskt = shared.pop("_maskt")[:n_w]

    in_maps = []
    for c in range(n_cores):
        m = dict(shared)
        m["mask"] = maskt
        m["x"] = np.ascontiguousarray(x[c % B, :n_w])
        in_maps.append(m)

    nc = _get_nc(n_w)
    res = run_bass_kernel_spmd(nc, in_maps, list(range(n_cores)), trace=_trace,
                               tmpdir=(os.environ.get("KERNEL_TRACE_DIR") if _trace else None))
    out = np.stack([res.results[c]["out"] for c in range(B)], axis=0)
    if _trace:
        kernel._last_exec_time_ns = res.exec_time_ns
        kernel._last_results = res
    return out



# revision 14
# speedup vs baseline: 1.3897x; 1.3897x over previous
"""Trainium2 Bass kernel for batched windowed multi-head attention.

Shapes: x (8, 64, 256, 512) f32, H=8 heads, D=64.
Sharding: data-parallel over batch dim B=8 -> 1 batch row per NeuronCore.
Each core processes 64 windows; per window a full MHA block computed in
fp32r (reduced-mantissa fp32, full-rate on the PE array):
  q/k/v projections, transposed scores = (k q^T) + pos_bias^T + mask^T,
  softmax along the PSUM partition axis: exp on ACT, denominators via a
  ones-column appended to V (so z_aug row 64 = sum_j exp), per-head
  normalization via a K=1 PE broadcast matmul + fast DVE reciprocal +
  one DVE multiply, out = z @ Wp^T + bp.
Windows are software-pipelined: projection chunks of window w+1 are
emitted interleaved with the attention heads of window w to keep the
PE dense (HAM clock stays warm).
"""
import os
import numpy as np
import ml_dtypes

import concourse.bass as bass
import concourse.mybir as mybir
import concourse.tile as tile
from concourse import bacc
from concourse.bass_utils import run_bass_kernel_spmd
from concourse.masks import make_identity

B, W, S, E = 8, 64, 256, 512
H, D = 8, 64
SCALE = D ** -0.5
NCORES = 8
F32 = mybir.dt.float32
F32R = mybir.dt.float32r
BF16 = mybir.dt.bfloat16
BF16NP = ml_dtypes.bfloat16
AOp = mybir.AluOpType
AF = mybir.ActivationFunctionType


def _emit(nc, tc, ctx, n_w, d):
    """Emit the per-core program: n_w windows of MHA."""
    const = ctx.enter_context(tc.tile_pool(name="const", bufs=1))

    # --- one-time: weights (bf16), biases, pos_bias, identity ---
    w_sb = {}
    for name in ("wq", "wk", "wv", "wp"):
        t = const.tile([128, 4, E], BF16, tag=name)
        nc.sync.dma_start(t[:], d[name].rearrange("(ic p) o -> p ic o", p=128))
        w_sb[name] = t

    bqc = const.tile([128, 4], F32)
    nc.sync.dma_start(bqc[:], d["bq"][:])
    bkc = const.tile([128, 4], F32)
    nc.sync.dma_start(bkc[:], d["bk"][:])
    bv_bc = const.tile([128, E], F32)
    nc.sync.dma_start(bv_bc[:], d["bv"][:])
    bp_bc = const.tile([128, E], F32)
    nc.sync.dma_start(bp_bc[:], d["bp"][:])

    # pos_bias TRANSPOSED per head: [128 (j%128), h, jc, i]
    pos_sb = const.tile([128, H, 2, S], F32)
    nc.sync.dma_start(pos_sb[:], d["pos"].rearrange("h (c p) j -> p h c j", p=128))

    ident = const.tile([128, 128], BF16)
    make_identity(nc, ident[:])
    ones16 = const.tile([128, 2, 8, 1], F32)
    nc.gpsimd.memset(ones16[:], 1.0)
    sel2 = const.tile([2, 128], BF16)
    nc.sync.dma_start(sel2[:], d["sel2"][:])

    # --- pools for the per-window pipeline ---
    xnat_p = ctx.enter_context(tc.tile_pool(name="xnat", bufs=2))
    msk_p = ctx.enter_context(tc.tile_pool(name="msk", bufs=2))
    mpb_p = ctx.enter_context(tc.tile_pool(name="mpb", bufs=2))
    xt_p = ctx.enter_context(tc.tile_pool(name="xt", bufs=2))
    qkv_p = ctx.enter_context(tc.tile_pool(name="qkv", bufs=2))
    zt_p = ctx.enter_context(tc.tile_pool(name="zt", bufs=2))
    outs_p = ctx.enter_context(tc.tile_pool(name="outs", bufs=2))
    attn_p = ctx.enter_context(tc.tile_pool(name="attn", bufs=4))
    expt_p = ctx.enter_context(tc.tile_pool(name="expt", bufs=4))
    den_p = ctx.enter_context(tc.tile_pool(name="den", bufs=8))

    ps_pj = ctx.enter_context(tc.tile_pool(name="ps_pj", bufs=3, space="PSUM"))
    ps_sc = ctx.enter_context(tc.tile_pool(name="ps_sc", bufs=2, space="PSUM"))
    ps_z = ctx.enter_context(tc.tile_pool(name="ps_z", bufs=3, space="PSUM"))

    def phase_a(w):
        """Load, transpose, and project window w (dense PE work)."""
        # load x window [256, 512] as [128, (s-chunk, e)]
        xnat = xnat_p.tile([128, 2, E], BF16, tag="xn", name=f"xn{w}")
        nc.sync.dma_start(xnat[:], d["x"][w].rearrange("(c p) e -> p c e", p=128))
        # mask^T window: [128 (j%128), jc, i]
        mskT = msk_p.tile([128, 2, S], F32, tag="mk", name=f"mk{w}")
        nc.sync.dma_start(mskT[:], d["mask"][w].rearrange("(c p) j -> p c j", p=128))

        # mask^T + pos_bias^T per head (gpsimd, sbuf only)
        mpbT = mpb_p.tile([128, H, 2, S], F32, tag="mpb", name=f"mpb{w}")
        for h in range(H):
            nc.gpsimd.tensor_tensor(mpbT[:, h], mskT[:], pos_sb[:, h], AOp.add)

        # xT [e, s] via PE transposes: [128 (e%128), (ec, s)]
        xT = xt_p.tile([128, 4, S], BF16, tag="xT", name=f"xT{w}")
        for ec in range(4):
            pt = ps_pj.tile([128, 2, 128], BF16, tag="pj", name=f"pt{w}_{ec}")
            for c in range(2):
                nc.tensor.transpose(pt[:, c], xnat[:, c, ec * 128:(ec + 1) * 128], ident[:])
            nc.vector.tensor_copy(xT[:, ec], pt[:])

        # projections: qT/kT [o, s] layout [128 (o%128), (oc, s)]
        qT = qkv_p.tile([128, 4, S], BF16, tag="qT", name=f"qT{w}")
        kT = qkv_p.tile([128, 4, S], BF16, tag="kT", name=f"kT{w}")
        vA = qkv_p.tile([128, 2, H, 65], BF16, tag="vA", name=f"vA{w}")
        nc.vector.tensor_copy(vA[:, :, :, 64:65], ones16[:])

        def qk_chunk(oc, wt, dst, bias):
            p = ps_pj.tile([128, S], F32, tag="pj", name=f"pp{w}_{wt}_{oc}")
            for ic in range(4):
                nc.tensor.matmul(p[:], w_sb[wt][:, ic, oc * 128:(oc + 1) * 128],
                                 xT[:, ic], start=(ic == 0), stop=(ic == 3))
            nc.scalar.activation(dst[:, oc], p[:], AF.Identity,
                                 bias=bias[:, oc:oc + 1])

        def v_chunk(sc):
            pv = ps_pj.tile([128, E], F32, tag="pj", name=f"pv{w}_{sc}")
            for ic in range(4):
                nc.tensor.matmul(pv[:], xT[:, ic, sc * 128:(sc + 1) * 128],
                                 w_sb["wv"][:, ic], start=(ic == 0), stop=(ic == 3))
            nc.vector.scalar_tensor_tensor(
                vA[:, sc, :, 0:64], pv[:].rearrange("p (h o) -> p h o", h=H),
                0.0, bv_bc[:].rearrange("p (h o) -> p h o", h=H),
                AOp.bypass, AOp.add)

        chunks = []
        for oc in range(4):
            chunks.append(lambda oc=oc: qk_chunk(oc, "wq", qT, bqc))
            chunks.append(lambda oc=oc: qk_chunk(oc, "wk", kT, bkc))
        chunks.append(lambda: v_chunk(0))
        chunks.append(lambda: v_chunk(1))
        return (qT, kT, vA, mpbT), chunks

    def phase_b(w, qT, kT, vA, mpbT):
        """Attention + output projection for window w (latency-heavy chain)."""
        # attention per head; zT [e, s] layout [128 (e%128), (hp, s)]
        zT = zt_p.tile([128, 4, S], BF16, tag="zT", name=f"zT{w}")

        def head(h):
            oc, prow = h // 2, (h % 2) * 64
            # transposed scores: [128 (j%128), jc, i]
            sT = ps_sc.tile([128, 2, S], F32, tag="sc", name=f"sT{w}_{h}")
            for jc in range(2):
                nc.tensor.matmul(sT[:, jc],
                                 kT[prow:prow + 64, oc, jc * 128:(jc + 1) * 128],
                                 qT[prow:prow + 64, oc], start=True, stop=True)
            astT = attn_p.tile([128, 2, S], F32, tag="astT", name=f"astT{w}_{h}")
            nc.vector.scalar_tensor_tensor(astT[:], sT[:], 0.0, mpbT[:, h],
                                           AOp.bypass, AOp.add)
            expT = expt_p.tile([128, 2, S], BF16, tag="expT", name=f"expT{w}_{h}")
            nc.scalar.activation(expT[:], astT[:], AF.Exp)
            # z_aug [65, i]: rows 0-63 = v_h^T @ exp cols, row 64 = sum_j exp
            za = ps_z.tile([65, S], F32, tag="zz", name=f"za{w}_{h}")
            for jc in range(2):
                nc.tensor.matmul(za[:], vA[:, jc, h], expT[:, jc],
                                 start=(jc == 0), stop=(jc == 1))
            den = den_p.tile([1, S], BF16, tag="den", name=f"den{w}_{h}")
            nc.scalar.copy(den[:], za[64:65, :])
            den_b = ps_z.tile([64, S], F32, tag="zz", name=f"denb{w}_{h}")
            nc.tensor.matmul(den_b[:], sel2[0:1, 0:64], den[:], start=True, stop=True)
            rec_b = den_p.tile([64, S], F32, tag="recb", name=f"recb{w}_{h}")
            nc.vector.reciprocal_approx_fast(rec_b[:], den_b[:])
            nc.vector.tensor_tensor(zT[prow:prow + 64, h // 2], za[0:64, :],
                                    rec_b[:], AOp.mult)

        def tail():
            # output projection [s, o] natural + bias, then store
            out_sb = outs_p.tile([128, 2, E], F32, tag="osb", name=f"osb{w}")
            for sc in range(2):
                po = ps_pj.tile([128, E], F32, tag="pj", name=f"po{w}_{sc}")
                for ec in range(4):
                    nc.tensor.matmul(po[:], zT[:, ec, sc * 128:(sc + 1) * 128],
                                     w_sb["wp"][:, ec], start=(ec == 0), stop=(ec == 3))
                nc.vector.scalar_tensor_tensor(out_sb[:, sc], po[:], 0.0, bp_bc[:],
                                               AOp.bypass, AOp.add)
            nc.sync.dma_start(d["out"][w].rearrange("(c p) e -> p c e", p=128), out_sb[:])

        return [lambda h=h: head(h) for h in range(H)], tail

    prev = None
    for w in range(n_w):
        cur, chunks = phase_a(w)
        if prev is not None:
            # interleave: one projection chunk of window w between heads of w-1
            heads, tail = phase_b(w - 1, *prev)
            seq = []
            ci = 0
            for hfn in heads:
                if ci < len(chunks):
                    seq.append(chunks[ci]); ci += 1
                seq.append(hfn)
            seq.extend(chunks[ci:])
            seq.append(tail)
            for fn in seq:
                fn()
        else:
            for fn in chunks:
                fn()
        prev = cur
    heads, tail = phase_b(n_w - 1, *prev)
    for fn in heads:
        fn()
    tail()


def _build(n_w):
    nc = bacc.Bacc("TRN2", target_bir_lowering=False, debug=False)
    d = {
        "x": nc.dram_tensor("x", [n_w, S, E], BF16, kind="ExternalInput"),
        "mask": nc.dram_tensor("mask", [n_w, S, S], F32, kind="ExternalInput"),
        "pos": nc.dram_tensor("pos", [H, S, S], F32, kind="ExternalInput"),
        "wq": nc.dram_tensor("wq", [E, E], BF16, kind="ExternalInput"),
        "wk": nc.dram_tensor("wk", [E, E], BF16, kind="ExternalInput"),
        "wv": nc.dram_tensor("wv", [E, E], BF16, kind="ExternalInput"),
        "wp": nc.dram_tensor("wp", [E, E], BF16, kind="ExternalInput"),
        "bq": nc.dram_tensor("bq", [128, 4], F32, kind="ExternalInput"),
        "bk": nc.dram_tensor("bk", [128, 4], F32, kind="ExternalInput"),
        "bv": nc.dram_tensor("bv", [128, E], F32, kind="ExternalInput"),
        "bp": nc.dram_tensor("bp", [128, E], F32, kind="ExternalInput"),
        "sel2": nc.dram_tensor("sel2", [2, 128], BF16, kind="ExternalInput"),
        "out": nc.dram_tensor("out", [n_w, S, E], F32, kind="ExternalOutput"),
    }
    from contextlib import ExitStack
    with tile.TileContext(nc) as tc, ExitStack() as ctx:
        _emit(nc, tc, ctx, n_w, d)
    nc.compile()
    return nc


_NC_CACHE = {}


def _get_nc(n_w):
    if n_w not in _NC_CACHE:
        _NC_CACHE[n_w] = _build(n_w)
    return _NC_CACHE[n_w]


def _host_prep(mask, Wq, bq, Wk, bk, Wv, bv, Wp, bp, pos_bias):
    """Shared (replicated) input tensors, host-side layout prep."""
    f = np.float32
    wq_t = np.ascontiguousarray((np.asarray(Wq, f).T * SCALE).astype(BF16NP))
    wk_t = np.ascontiguousarray(np.asarray(Wk, f).T.astype(BF16NP))
    wv_t = np.ascontiguousarray(np.asarray(Wv, f).T.astype(BF16NP))
    wp_t = np.ascontiguousarray(np.asarray(Wp, f).T.astype(BF16NP))
    bq_s = (bq * SCALE).astype(f)
    # bias tiles for qT/kT layout: [128 (o%128), oc, s] broadcast along s
    bq_t = np.ascontiguousarray(bq_s.reshape(4, 128).T)
    bk_t = np.ascontiguousarray(np.asarray(bk, f).reshape(4, 128).T)
    bv_bc = np.ascontiguousarray(np.broadcast_to(np.asarray(bv, f)[None, :], (128, E)))
    bp_bc = np.ascontiguousarray(np.broadcast_to(np.asarray(bp, f)[None, :], (128, E)))
    # transposed mask / pos_bias for the partition-axis softmax layout
    maskt = np.ascontiguousarray(np.asarray(mask, f)[0, :, 0].transpose(0, 2, 1))
    sel2 = np.ascontiguousarray((np.arange(128)[None, :] // 64 == np.arange(2)[:, None]).astype(BF16NP))
    post = np.ascontiguousarray(np.asarray(pos_bias, f).transpose(0, 2, 1))
    return {
        "wq": wq_t, "wk": wk_t, "wv": wv_t, "wp": wp_t,
        "bq": bq_t, "bk": bk_t, "bv": bv_bc, "bp": bp_bc,
        "pos": post, "_maskt": maskt,
        "sel2": sel2,
    }


def _make_in_maps(x, mask, Wq, bq, Wk, bk, Wv, bv, Wp, bp, pos_bias, n_w, n_cores):
    x = np.asarray(x, np.float32).astype(BF16NP)
    shared = _host_prep(mask, Wq, bq, Wk, bk, Wv, bv, Wp, bp, pos_bias)
    maskt = shared.pop("_maskt")[:n_w]

    in_maps = []
    for c in range(n_cores):
        m = dict(shared)
        m["mask"] = maskt
        m["x"] = np.ascontiguousarray(x[c % B, :n_w])
        in_maps.append(m)
    return in_maps


def kernel(x, mask, Wq, bq, Wk, bk, Wv, bv, Wp, bp, pos_bias, _trace=False):
    n_w = int(os.environ.get("KERNEL_NW", W))
    n_cores = NCORES
    in_maps = _make_in_maps(x, mask, Wq, bq, Wk, bk, Wv, bv, Wp, bp, pos_bias,
                            n_w, n_cores)

    nc = _get_nc(n_w)
    res = run_bass_kernel_spmd(nc, in_maps, list(range(n_cores)), trace=_trace,
                               tmpdir=(os.environ.get("KERNEL_TRACE_DIR") if _trace else None))
    out = np.stack([res.results[c]["out"] for c in range(B)], axis=0)
    if _trace:
        kernel._last_exec_time_ns = res.exec_time_ns
        kernel._last_results = res
    return out



# revision 15
# speedup vs baseline: 1.4889x; 1.0714x over previous
"""Trainium2 Bass kernel for batched windowed multi-head attention.

Shapes: x (8, 64, 256, 512) f32, H=8 heads, D=64.
Sharding: data-parallel over batch dim B=8 -> 1 batch row per NeuronCore.
Each core processes 64 windows; per window a full MHA block in bf16 operands
with fp32 PSUM accumulation:
  - xT loaded directly via DMA xbar transpose (no PE transposes),
  - q/k/v projections with bf16 weights (LDWEIGHTS at 1 cyc/col),
  - transposed scores sT = k q^T per head; softmax via
    exp(s)*exp(mask+pos): exp(mask) precomputed host-side, exp(pos) const,
    em = emask*epos on GPSIMD, pexp = exp(sT)*em on DVE (all bf16),
  - z_aug = [v; 1]^T @ pexp gives z rows + denominator row; heads are
    processed in groups of 4 so score matmuls pair across PE row-groups
    (concurrent K=64 matmuls) and two heads share each PSUM bank for the
    denominator broadcast / reciprocal / normalize ops ([64,512] DVE ops),
  - out = z @ Wp^T + bp, f32 out.
Windows are software-pipelined: projection chunks of window w+1 are
interleaved with the attention head groups of window w.
"""
import os
import numpy as np
import ml_dtypes

import concourse.bass as bass
import concourse.mybir as mybir
import concourse.tile as tile
from concourse import bacc
from concourse.bass_utils import run_bass_kernel_spmd

B, W, S, E = 8, 64, 256, 512
H, D = 8, 64
SCALE = D ** -0.5
NCORES = 8
F32 = mybir.dt.float32
BF16 = mybir.dt.bfloat16
BF16NP = ml_dtypes.bfloat16
AOp = mybir.AluOpType
AF = mybir.ActivationFunctionType


def _emit(nc, tc, ctx, n_w, d):
    """Emit the per-core program: n_w windows of MHA."""
    const = ctx.enter_context(tc.tile_pool(name="const", bufs=1))

    # --- one-time constants ---
    w_sb = {}
    for name in ("wq", "wk", "wv", "wp"):
        t = const.tile([128, 4, E], BF16, tag=name)
        nc.sync.dma_start(t[:], d[name].rearrange("(ic p) o -> p ic o", p=128))
        w_sb[name] = t

    bqc = const.tile([128, 4], F32)
    nc.sync.dma_start(bqc[:], d["bq"][:])
    bkc = const.tile([128, 4], F32)
    nc.sync.dma_start(bkc[:], d["bk"][:])
    bv_bc = const.tile([128, E], F32)
    nc.sync.dma_start(bv_bc[:], d["bv"][:])
    bp_bc = const.tile([128, E], F32)
    nc.sync.dma_start(bp_bc[:], d["bp"][:])

    # exp(pos_bias)^T per head: [128 (j%128), h, jc, i] bf16
    epos = const.tile([128, H, 2, S], BF16)
    nc.sync.dma_start(epos[:], d["pos"].rearrange("h (c p) j -> p h c j", p=128))

    ones16 = const.tile([128, 2, 8, 1], F32)
    nc.gpsimd.memset(ones16[:], 1.0)
    sel2 = const.tile([2, 128], BF16)
    nc.sync.dma_start(sel2[:], d["sel2"][:])

    # --- pools for the per-window pipeline ---
    emsk_p = ctx.enter_context(tc.tile_pool(name="emsk", bufs=2))
    em_p = ctx.enter_context(tc.tile_pool(name="em", bufs=2))
    xt_p = ctx.enter_context(tc.tile_pool(name="xt", bufs=2))
    qkv_p = ctx.enter_context(tc.tile_pool(name="qkv", bufs=2))
    zt_p = ctx.enter_context(tc.tile_pool(name="zt", bufs=2))
    outs_p = ctx.enter_context(tc.tile_pool(name="outs", bufs=2))
    exp_p = ctx.enter_context(tc.tile_pool(name="exp", bufs=4))
    pexp_p = ctx.enter_context(tc.tile_pool(name="pexp", bufs=4))
    den_p = ctx.enter_context(tc.tile_pool(name="den", bufs=4))
    rec_p = ctx.enter_context(tc.tile_pool(name="rec", bufs=2))

    ps_pj = ctx.enter_context(tc.tile_pool(name="ps_pj", bufs=2, space="PSUM"))
    ps_sc = ctx.enter_context(tc.tile_pool(name="ps_sc", bufs=3, space="PSUM"))
    ps_z = ctx.enter_context(tc.tile_pool(name="ps_z", bufs=2, space="PSUM"))
    ps_db = ctx.enter_context(tc.tile_pool(name="ps_db", bufs=1, space="PSUM"))

    def phase_a(w):
        """Load + project window w (dense PE work), em precompute (gpsimd)."""
        # xT [e, s] via DMA xbar transpose: [128 (e%128), ec, s] bf16
        xT = xt_p.tile([128, 4, S], BF16, tag="xT", name=f"xT{w}")
        nc.sync.dma_start_transpose(xT[:], d["x"][w])
        # exp(mask)^T window: [128 (j%128), jc, i] bf16
        emsk = emsk_p.tile([128, 2, S], BF16, tag="mk", name=f"mk{w}")
        nc.sync.dma_start(emsk[:], d["mask"][w].rearrange("(c p) j -> p c j", p=128))

        # em = exp(mask)^T * exp(pos)^T per head (gpsimd, sbuf bf16)
        em = em_p.tile([128, H, 2, S], BF16, tag="em", name=f"em{w}")
        for h in range(H):
            nc.gpsimd.tensor_tensor(em[:, h], emsk[:], epos[:, h], AOp.mult)

        qT = qkv_p.tile([128, 4, S], BF16, tag="qT", name=f"qT{w}")
        kT = qkv_p.tile([128, 4, S], BF16, tag="kT", name=f"kT{w}")
        vA = qkv_p.tile([128, 2, H, 65], BF16, tag="vA", name=f"vA{w}")
        nc.vector.tensor_copy(vA[:, :, :, 64:65], ones16[:])

        def qk_chunk(oc, wt, dst, bias, on_act):
            p = ps_pj.tile([128, S], F32, tag="pj", name=f"pp{w}_{wt}_{oc}")
            for ic in range(4):
                nc.tensor.matmul(p[:], w_sb[wt][:, ic, oc * 128:(oc + 1) * 128],
                                 xT[:, ic], start=(ic == 0), stop=(ic == 3))
            if on_act:
                nc.scalar.activation(dst[:, oc], p[:], AF.Identity,
                                     bias=bias[:, oc:oc + 1])
            else:
                nc.vector.tensor_scalar(dst[:, oc], p[:], bias[:, oc:oc + 1],
                                        None, AOp.add)

        def v_chunk(sc):
            pv = ps_pj.tile([128, E], F32, tag="pj", name=f"pv{w}_{sc}")
            for ic in range(4):
                nc.tensor.matmul(pv[:], xT[:, ic, sc * 128:(sc + 1) * 128],
                                 w_sb["wv"][:, ic], start=(ic == 0), stop=(ic == 3))
            nc.vector.scalar_tensor_tensor(
                vA[:, sc, :, 0:64], pv[:].rearrange("p (h o) -> p h o", h=H),
                0.0, bv_bc[:].rearrange("p (h o) -> p h o", h=H),
                AOp.bypass, AOp.add)

        chunks = []
        for oc in range(4):
            chunks.append(lambda oc=oc: qk_chunk(oc, "wq", qT, bqc, True))
            chunks.append(lambda oc=oc: qk_chunk(oc, "wk", kT, bkc, False))
        chunks.append(lambda: v_chunk(0))
        chunks.append(lambda: v_chunk(1))
        return (qT, kT, vA, em), chunks

    def phase_b(w, qT, kT, vA, em):
        """Attention + output projection for window w, as interleavable atoms."""
        zT = zt_p.tile([128, 4, S], BF16, tag="zT", name=f"zT{w}")

        def head_group(g4):
            """Heads 4*g4 .. 4*g4+3: paired scores, shared-bank softmax."""
            hs = [4 * g4 + i for i in range(4)]
            sT = {}
            pexp = {}

            def scores(hpair):
                # alternating row groups (prow 0 / 64) for PE concurrency
                for jc in range(2):
                    for h in hpair:
                        oc, prow = h // 2, (h % 2) * 64
                        if h not in sT:
                            sT[h] = ps_sc.tile([128, 2, S], F32, tag="sc",
                                               name=f"sT{w}_{h}")
                        nc.tensor.matmul(
                            sT[h][:, jc],
                            kT[prow:prow + 64, oc, jc * 128:(jc + 1) * 128],
                            qT[prow:prow + 64, oc], start=True, stop=True)

            def soft(h):
                e0 = exp_p.tile([128, 2, S], BF16, tag="e0", name=f"e0{w}_{h}")
                nc.scalar.activation(e0[:], sT[h][:], AF.Exp)
                pexp[h] = pexp_p.tile([128, 2, S], BF16, tag="px",
                                      name=f"px{w}_{h}")
                nc.vector.tensor_tensor(pexp[h][:], e0[:], em[:, h], AOp.mult)

            def zmm(hpair2, zp):
                # two heads (same prow, adjacent oc) share one PSUM bank
                za = ps_z.tile([65, 2, S], F32, tag="zz", name=f"za{w}_{zp}")
                first = True
                for col, h in enumerate(hpair2):
                    for jc in range(2):
                        nc.tensor.matmul(za[:, col], vA[:, jc, h], pexp[h][:, jc],
                                         start=first, stop=(col == 1 and jc == 1),
                                         skip_group_check=True)
                        first = False
                return za

            def norm(hpair2, za):
                prow = (hpair2[0] % 2) * 64
                oc0 = hpair2[0] // 2
                den2 = den_p.tile([1, 2, S], BF16, tag="den",
                                  name=f"den{w}_{hpair2[0]}")
                nc.scalar.copy(den2[:], za[64:65])
                db = ps_db.tile([64, 2, S], F32, tag="db",
                                name=f"db{w}_{hpair2[0]}")
                nc.tensor.matmul(db[:], sel2[0:1, 0:64],
                                 den2[:].rearrange("p c s -> p (c s)"),
                                 start=True, stop=True)
                rec = rec_p.tile([64, 2, S], F32, tag="rec",
                                 name=f"rec{w}_{hpair2[0]}")
                nc.vector.reciprocal_approx_fast(rec[:], db[:])
                nc.vector.tensor_tensor(zT[prow:prow + 64, oc0:oc0 + 2],
                                        za[0:64], rec[:], AOp.mult)

            atoms = []
            atoms.append(lambda: scores(hs[0:2]))
            atoms.append(lambda: (soft(hs[0]), soft(hs[1])))
            atoms.append(lambda: scores(hs[2:4]))
            atoms.append(lambda: (soft(hs[2]), soft(hs[3])))
            # pairs sharing prow: (h0, h2) and (h1, h3)
            pa = (hs[0], hs[2])
            pb = (hs[1], hs[3])
            st = {}
            atoms.append(lambda: st.__setitem__("a", zmm(pa, f"a{g4}")))
            atoms.append(lambda: norm(pa, st["a"]))
            atoms.append(lambda: st.__setitem__("b", zmm(pb, f"b{g4}")))
            atoms.append(lambda: norm(pb, st["b"]))
            return atoms

        def tail():
            # output projection [s, o] natural + bias, then store
            out_sb = outs_p.tile([128, 2, E], F32, tag="osb", name=f"osb{w}")
            for sc in range(2):
                po = ps_pj.tile([128, E], F32, tag="pj", name=f"po{w}_{sc}")
                for ec in range(4):
                    nc.tensor.matmul(po[:], zT[:, ec, sc * 128:(sc + 1) * 128],
                                     w_sb["wp"][:, ec], start=(ec == 0), stop=(ec == 3))
                nc.vector.scalar_tensor_tensor(out_sb[:, sc], po[:], 0.0, bp_bc[:],
                                               AOp.bypass, AOp.add)
            nc.sync.dma_start(d["out"][w].rearrange("(c p) e -> p c e", p=128), out_sb[:])

        atoms = head_group(0) + head_group(1)
        return atoms, tail

    prev = None
    for w in range(n_w):
        cur, chunks = phase_a(w)
        if prev is not None:
            # interleave: projection chunks of window w between atoms of w-1
            atoms, tail = phase_b(w - 1, *prev)
            seq = []
            ci = 0
            for afn in atoms:
                seq.append(afn)
                if ci < len(chunks):
                    seq.append(chunks[ci]); ci += 1
                if ci < len(chunks) and len(seq) % 3 == 2:
                    seq.append(chunks[ci]); ci += 1
            seq.extend(chunks[ci:])
            seq.append(tail)
            for fn in seq:
                fn()
        else:
            for fn in chunks:
                fn()
        prev = cur
    atoms, tail = phase_b(n_w - 1, *prev)
    for fn in atoms:
        fn()
    tail()


def _build(n_w):
    nc = bacc.Bacc("TRN2", target_bir_lowering=False, debug=False)
    d = {
        "x": nc.dram_tensor("x", [n_w, S, E], BF16, kind="ExternalInput"),
        "mask": nc.dram_tensor("mask", [n_w, S, S], BF16, kind="ExternalInput"),
        "pos": nc.dram_tensor("pos", [H, S, S], BF16, kind="ExternalInput"),
        "wq": nc.dram_tensor("wq", [E, E], BF16, kind="ExternalInput"),
        "wk": nc.dram_tensor("wk", [E, E], BF16, kind="ExternalInput"),
        "wv": nc.dram_tensor("wv", [E, E], BF16, kind="ExternalInput"),
        "wp": nc.dram_tensor("wp", [E, E], BF16, kind="ExternalInput"),
        "bq": nc.dram_tensor("bq", [128, 4], F32, kind="ExternalInput"),
        "bk": nc.dram_tensor("bk", [128, 4], F32, kind="ExternalInput"),
        "bv": nc.dram_tensor("bv", [128, E], F32, kind="ExternalInput"),
        "bp": nc.dram_tensor("bp", [128, E], F32, kind="ExternalInput"),
        "sel2": nc.dram_tensor("sel2", [2, 128], BF16, kind="ExternalInput"),
        "out": nc.dram_tensor("out", [n_w, S, E], F32, kind="ExternalOutput"),
    }
    from contextlib import ExitStack
    with tile.TileContext(nc) as tc, ExitStack() as ctx:
        _emit(nc, tc, ctx, n_w, d)
    nc.compile()
    return nc


_NC_CACHE = {}


def _get_nc(n_w):
    if n_w not in _NC_CACHE:
        _NC_CACHE[n_w] = _build(n_w)
    return _NC_CACHE[n_w]


def _host_prep(mask, Wq, bq, Wk, bk, Wv, bv, Wp, bp, pos_bias):
    """Shared (replicated) input tensors, host-side layout prep."""
    f = np.float32
    wq_t = np.ascontiguousarray((np.asarray(Wq, f).T * SCALE).astype(BF16NP))
    wk_t = np.ascontiguousarray(np.asarray(Wk, f).T.astype(BF16NP))
    wv_t = np.ascontiguousarray(np.asarray(Wv, f).T.astype(BF16NP))
    wp_t = np.ascontiguousarray(np.asarray(Wp, f).T.astype(BF16NP))
    bq_s = (bq * SCALE).astype(f)
    # bias tiles for qT/kT layout: [128 (o%128), oc, s] broadcast along s
    bq_t = np.ascontiguousarray(bq_s.reshape(4, 128).T)
    bk_t = np.ascontiguousarray(np.asarray(bk, f).reshape(4, 128).T)
    bv_bc = np.ascontiguousarray(np.broadcast_to(np.asarray(bv, f)[None, :], (128, E)))
    bp_bc = np.ascontiguousarray(np.broadcast_to(np.asarray(bp, f)[None, :], (128, E)))
    # exp of transposed mask / pos_bias for the partition-axis softmax layout
    emaskt = np.ascontiguousarray(
        np.exp(np.asarray(mask, f)[0, :, 0].transpose(0, 2, 1)).astype(BF16NP))
    sel2 = np.ascontiguousarray((np.arange(128)[None, :] // 64 == np.arange(2)[:, None]).astype(BF16NP))
    epost = np.ascontiguousarray(
        np.exp(np.asarray(pos_bias, f).transpose(0, 2, 1)).astype(BF16NP))
    return {
        "wq": wq_t, "wk": wk_t, "wv": wv_t, "wp": wp_t,
        "bq": bq_t, "bk": bk_t, "bv": bv_bc, "bp": bp_bc,
        "pos": epost, "_maskt": emaskt,
        "sel2": sel2,
    }


def _make_in_maps(x, mask, Wq, bq, Wk, bk, Wv, bv, Wp, bp, pos_bias, n_w, n_cores):
    x = np.asarray(x, np.float32).astype(BF16NP)
    shared = _host_prep(mask, Wq, bq, Wk, bk, Wv, bv, Wp, bp, pos_bias)
    maskt = shared.pop("_maskt")[:n_w]

    in_maps = []
    for c in range(n_cores):
        m = dict(shared)
        m["mask"] = maskt
        m["x"] = np.ascontiguousarray(x[c % B, :n_w])
        in_maps.append(m)
    return in_maps


def kernel(x, mask, Wq, bq, Wk, bk, Wv, bv, Wp, bp, pos_bias, _trace=False):
    n_w = int(os.environ.get("KERNEL_NW", W))
    n_cores = NCORES
    in_maps = _make_in_maps(x, mask, Wq, bq, Wk, bk, Wv, bv, Wp, bp, pos_bias,
                            n_w, n_cores)

    nc = _get_nc(n_w)
    res = run_bass_kernel_spmd(nc, in_maps, list(range(n_cores)), trace=_trace,
                               tmpdir=(os.environ.get("KERNEL_TRACE_DIR") if _trace else None))
    out = np.stack([res.results[c]["out"] for c in range(B)], axis=0)
    if _trace:
        kernel._last_exec_time_ns = res.exec_time_ns
        kernel._last_results = res
    return out
